# revision 6
# baseline (speedup 1.0000x reference)
"""DSS ('softmax' variant, arxiv 2203.14343) kernel for Trainium2.

Computes K[c,h,l] = Re( sum_n Wc[c,h,n] * exp(dt_Lambda[h,n] * l - P_max[h,n]) )
with the structured-softmax normalization of Wc, for
C=1, H=128, N=64, L=8192, sharded over H across 8 NeuronCores.

Math used on device, per core (16 h-channels -> 8 groups of 2h x 64n = 128
partitions):
  a[h,n]   = dt_re[h]*Lam_re[n] + i*dt_im[h]*Lam_im[n]      (dt_Lambda)
  l = j*B + b   (B=128, J=64)
  S[h,n,l] = exp(a*l - pmax) = exp(a*B*j - pmax) * exp(a*b)
  K[h,l]   = Re( sum_n (Wc * exp(a*B*j - pmax)) * exp(a*b) )
           = sum_n U_re*V_re - U_im*V_im        <- two PE matmul-accumulates
                                                   (contraction over n)
All transcendentals (exp/sin) on the scalar (ACT) engine, batched by table
set; elementwise combines on the vector engine with broadcast APs; the n-sum
on the tensor engine; PSUM -> SBUF -> HBM for the output.
"""
import numpy as np
from contextlib import ExitStack

H_DIM = 128
N_STATE = 64
SEQ_LEN = 8192
NCORES = 8
HLOC = H_DIM // NCORES      # 16 channels per core
G = HLOC // 2               # 8 groups of (2h x 64n) partitions
B = 128                     # intra-block length (matmul moving free dim)
J = SEQ_LEN // B            # 64 blocks (matmul stationary free dim / M)
P = 128
EPS = 1e-7

_COMPILED = {}


def _build():
    import concourse.bacc as bacc
    import concourse.tile as tile
    import concourse.mybir as mybir

    f32 = mybir.dt.float32
    AF = mybir.ActivationFunctionType
    ALU = mybir.AluOpType

    nc = bacc.Bacc("TRN2", target_bir_lowering=False, debug=False,
                   num_devices=NCORES)
    lam_in = nc.dram_tensor("lam_b", [P, 2], f32, kind="ExternalInput").ap()
    ldt_in = nc.dram_tensor("ldt_b", [P, 2 * G], f32, kind="ExternalInput").ap()
    w_in = nc.dram_tensor("w_b", [P, 2 * G], f32, kind="ExternalInput").ap()
    k_out = nc.dram_tensor("k_out", [HLOC, SEQ_LEN], f32,
                           kind="ExternalOutput").ap()

    with tile.TileContext(nc) as tc, ExitStack() as ctx:
        pool = ctx.enter_context(tc.tile_pool(name="main", bufs=1))
        ppool = ctx.enter_context(tc.tile_pool(name="ps", bufs=G, space="PSUM"))
        kpool = ctx.enter_context(tc.tile_pool(name="ksb", bufs=G))

        def t(shape, tag):
            return pool.tile(shape, f32, tag=tag, name=tag)

        def v3(ap, inner):
            return ap.rearrange("p (g x) -> p g x", x=inner)

        def bc(ap, inner):
            return ap.to_broadcast([P, G, inner])

        # ---- loads ----
        lam = t([P, 2], "lam")
        ldt = t([P, 2 * G], "ldt")
        wt = t([P, 2 * G], "wt")
        nc.sync.dma_start(lam[:, :], lam_in)
        nc.sync.dma_start(ldt[:, :], ldt_in)
        nc.sync.dma_start(wt[:, :], w_in)
        lam_re, lam_im = lam[:, 0:1], lam[:, 1:2]

        # ---- iotas ----
        iota_j = t([P, G * J], "iota_j")
        nc.gpsimd.iota(iota_j[:, :], pattern=[[0, G], [1, J]], base=0,
                       channel_multiplier=0,
                       allow_small_or_imprecise_dtypes=True)
        iota_b = t([P, G * B], "iota_b")
        nc.gpsimd.iota(iota_b[:, :], pattern=[[0, G], [1, B]], base=0,
                       channel_multiplier=0,
                       allow_small_or_imprecise_dtypes=True)
        halfpi = t([P, 1], "halfpi")
        nc.gpsimd.memset(halfpi[:, :], float(np.float32(np.pi / 2)))

        # ---- ACT Exp batch 1: dt = exp(log_dt) ----
        dt = t([P, 2 * G], "dt")
        nc.scalar.activation(dt[:, :], ldt[:, :], AF.Exp)
        dt_re = v3(dt[:, :], 2)[:, :, 0]
        dt_im = v3(dt[:, :], 2)[:, :, 1]

        # ---- per-(h,n) scalars, [P, G] ----
        a_re = t([P, G], "a_re")
        a_im = t([P, G], "a_im")
        nc.vector.tensor_scalar(a_re[:, :], dt_re, lam_re, None, ALU.mult)
        nc.vector.tensor_scalar(a_im[:, :], dt_im, lam_im, None, ALU.mult)
        gt0 = t([P, 1], "gt0")
        nc.vector.tensor_scalar(gt0[:, :], lam_re, 0.0, None, ALU.is_gt)
        sgn = t([P, 1], "sgn")
        nc.vector.tensor_scalar(sgn[:, :], gt0[:, :], -2.0, 1.0, ALU.mult,
                                ALU.add)
        dn_re = t([P, G], "dn_re")
        dn_im = t([P, G], "dn_im")
        nc.vector.tensor_scalar(dn_re[:, :], a_re[:, :], sgn[:, 0:1], None,
                                ALU.mult)
        nc.vector.tensor_scalar(dn_im[:, :], a_im[:, :], sgn[:, 0:1], None,
                                ALU.mult)
        pm_re = t([P, G], "pm_re")
        pm_im = t([P, G], "pm_im")
        nc.vector.tensor_scalar(pm_re[:, :], a_re[:, :], gt0[:, 0:1],
                                float(SEQ_LEN - 1), ALU.mult, ALU.mult)
        nc.vector.tensor_scalar(pm_im[:, :], a_im[:, :], gt0[:, 0:1],
                                float(SEQ_LEN - 1), ALU.mult, ALU.mult)
        a_reB = t([P, G], "a_reB")
        a_imB = t([P, G], "a_imB")
        nc.vector.tensor_scalar(a_reB[:, :], a_re[:, :], float(B), None,
                                ALU.mult)
        nc.vector.tensor_scalar(a_imB[:, :], a_im[:, :], float(B), None,
                                ALU.mult)

        # ---- phase tensors (DVE, broadcast APs) ----
        thu_re = t([P, G * J], "thu_re")
        thu_im = t([P, G * J], "thu_im")
        nc.vector.tensor_tensor(v3(thu_re[:, :], J), v3(iota_j[:, :], J),
                                bc(a_reB[:, :], J), ALU.mult)
        nc.vector.tensor_tensor(v3(thu_re[:, :], J), v3(thu_re[:, :], J),
                                bc(pm_re[:, :], J), ALU.subtract)
        nc.vector.tensor_tensor(v3(thu_im[:, :], J), v3(iota_j[:, :], J),
                                bc(a_imB[:, :], J), ALU.mult)
        nc.vector.tensor_tensor(v3(thu_im[:, :], J), v3(thu_im[:, :], J),
                                bc(pm_im[:, :], J), ALU.subtract)
        thv_re = t([P, G * B], "thv_re")
        thv_im = t([P, G * B], "thv_im")
        nc.vector.tensor_tensor(v3(thv_re[:, :], B), v3(iota_b[:, :], B),
                                bc(a_re[:, :], B), ALU.mult)
        nc.vector.tensor_tensor(v3(thv_im[:, :], B), v3(iota_b[:, :], B),
                                bc(a_im[:, :], B), ALU.mult)

        # ---- ACT Exp batch 2 ----
        e1 = t([P, G], "e1")
        eL = t([P, G], "eL")
        nc.scalar.activation(e1[:, :], dn_re[:, :], AF.Exp)
        nc.scalar.activation(eL[:, :], dn_re[:, :], AF.Exp,
                             scale=float(SEQ_LEN))
        Eu = t([P, G * J], "Eu")
        Ev = t([P, G * B], "Ev")
        nc.scalar.activation(Eu[:, :], thu_re[:, :], AF.Exp)
        nc.scalar.activation(Ev[:, :], thv_re[:, :], AF.Exp)

        # ---- range reduction for all sine/cosine arguments ----
        # ACT Sin is only accurate on roughly [-pi, pi] (no HW range
        # reduction), so reduce args mod 2*pi with a magic-round +
        # two-term Cody-Waite subtraction. The cosine argument gets its own
        # reduction shifted by a quarter turn so that (rc + pi/2) lands in
        # [-pi, pi] as well.
        TS = nc.vector.tensor_scalar
        STT = nc.vector.scalar_tensor_tensor
        M_MAGIC = float(np.float32(1.5 * 2 ** 23))
        INV2PI = float(np.float32(1.0 / (2 * np.pi)))
        NC1 = -6.28125  # 8-bit mantissa: k*c1 exact for k < 2^16
        NC2 = -float(np.float32(2 * np.pi - 6.28125))
        # clamp bounds: keep sin args (and rc + pi/2) strictly inside
        # [-pi, pi] for both the HW table and the simulator's range check
        PI_LO = float(np.float32(3.1415925))
        HPF = np.float32(np.pi / 2)
        RC_HI = float(np.float32(np.float32(PI_LO) - HPF))
        RC_LO = -float(np.float32(np.float32(PI_LO) + HPF))

        def reduce2(theta, w, name):
            """theta: [P, w] AP. Returns (rs, rc): sin(theta) = sin(rs),
            cos(theta) = sin(rc + pi/2)."""
            m = t([P, w], name + "_m")
            TS(m[:, :], theta, INV2PI, None, ALU.mult)
            k = t([P, w], name + "_k")
            TS(k[:, :], m[:, :], M_MAGIC, M_MAGIC, ALU.add, ALU.subtract)
            kc = t([P, w], name + "_kc")
            TS(kc[:, :], m[:, :], 0.25, None, ALU.add)
            TS(kc[:, :], kc[:, :], M_MAGIC, M_MAGIC, ALU.add, ALU.subtract)
            rs = t([P, w], name + "_rs")
            STT(rs[:, :], k[:, :], NC1, theta, ALU.mult, ALU.add)
            STT(rs[:, :], k[:, :], NC2, rs[:, :], ALU.mult, ALU.add)
            TS(rs[:, :], rs[:, :], PI_LO, -PI_LO, ALU.min, ALU.max)
            rc = t([P, w], name + "_rc")
            STT(rc[:, :], kc[:, :], NC1, theta, ALU.mult, ALU.add)
            STT(rc[:, :], kc[:, :], NC2, rc[:, :], ALU.mult, ALU.add)
            TS(rc[:, :], rc[:, :], RC_HI, RC_LO, ALU.min, ALU.max)
            return rs, rc

        dnL_im = t([P, G], "dnL_im")  # dn_im * L (exact: L is a power of 2)
        TS(dnL_im[:, :], dn_im[:, :], float(SEQ_LEN), None, ALU.mult)
        rs_d, rc_d = reduce2(dn_im[:, :], G, "rd")
        rs_dL, rc_dL = reduce2(dnL_im[:, :], G, "rdL")
        rs_u, rc_u = reduce2(thu_im[:, :], G * J, "ru")
        rs_v, rc_v = reduce2(thv_im[:, :], G * B, "rv")

        # ---- ACT Sin batch (cos via +pi/2 bias on the shifted reduction) ----
        c1 = t([P, G], "c1")
        s1 = t([P, G], "s1")
        cL = t([P, G], "cL")
        sL = t([P, G], "sL")
        nc.scalar.activation(c1[:, :], rc_d[:, :], AF.Sin, bias=halfpi[:, 0:1])
        nc.scalar.activation(s1[:, :], rs_d[:, :], AF.Sin)
        nc.scalar.activation(cL[:, :], rc_dL[:, :], AF.Sin,
                             bias=halfpi[:, 0:1])
        nc.scalar.activation(sL[:, :], rs_dL[:, :], AF.Sin)
        Cu = t([P, G * J], "Cu")
        Su = t([P, G * J], "Su")
        nc.scalar.activation(Cu[:, :], rc_u[:, :], AF.Sin,
                             bias=halfpi[:, 0:1])
        nc.scalar.activation(Su[:, :], rs_u[:, :], AF.Sin)
        Cv = t([P, G * B], "Cv")
        Sv = t([P, G * B], "Sv")  # sin(-theta) = -sin(theta): fold the matmul minus
        nc.scalar.activation(Cv[:, :], rc_v[:, :], AF.Sin,
                             bias=halfpi[:, 0:1])
        nc.scalar.activation(Sv[:, :], rs_v[:, :], AF.Sin, scale=-1.0)

        # ---- normalization coefficients Wc (all [P, G]) ----
        TT = nc.vector.tensor_tensor
        TS = nc.vector.tensor_scalar
        num_re = t([P, G], "num_re")
        num_im = t([P, G], "num_im")
        TT(num_re[:, :], e1[:, :], c1[:, :], ALU.mult)
        TS(num_re[:, :], num_re[:, :], -1.0, None, ALU.add)
        TT(num_im[:, :], e1[:, :], s1[:, :], ALU.mult)
        den_re = t([P, G], "den_re")
        den_im = t([P, G], "den_im")
        TT(den_re[:, :], eL[:, :], cL[:, :], ALU.mult)
        TS(den_re[:, :], den_re[:, :], -1.0, None, ALU.add)
        TT(den_im[:, :], eL[:, :], sL[:, :], ALU.mult)
        neg_lam_im = t([P, 1], "neg_lam_im")
        TS(neg_lam_im[:, :], lam_im, -1.0, None, ALU.mult)
        x_re = t([P, G], "x_re")
        x_im = t([P, G], "x_im")
        tmp1 = t([P, G], "tmp1")
        TS(tmp1[:, :], den_re[:, :], lam_re, None, ALU.mult)
        nc.vector.scalar_tensor_tensor(x_re[:, :], den_im[:, :],
                                       neg_lam_im[:, 0:1], tmp1[:, :],
                                       ALU.mult, ALU.add)
        TS(tmp1[:, :], den_im[:, :], lam_re, None, ALU.mult)
        nc.vector.scalar_tensor_tensor(x_im[:, :], den_re[:, :],
                                       lam_im, tmp1[:, :], ALU.mult, ALU.add)
        d = t([P, G], "d")
        TT(d[:, :], x_re[:, :], x_re[:, :], ALU.mult)
        TT(tmp1[:, :], x_im[:, :], x_im[:, :], ALU.mult)
        TT(d[:, :], d[:, :], tmp1[:, :], ALU.add)
        TS(d[:, :], d[:, :], float(EPS), None, ALU.add)
        inv = t([P, G], "inv")
        nc.vector.reciprocal(inv[:, :], d[:, :])
        rr = t([P, G], "rr")
        rim = t([P, G], "rim")  # rim = x_im*inv = -recip_im
        TT(rr[:, :], x_re[:, :], inv[:, :], ALU.mult)
        TT(rim[:, :], x_im[:, :], inv[:, :], ALU.mult)
        # q = num * recip   (recip = rr - i*rim)
        q_re = t([P, G], "q_re")
        q_im = t([P, G], "q_im")
        tmp2 = t([P, G], "tmp2")
        TT(tmp1[:, :], num_re[:, :], rr[:, :], ALU.mult)
        TT(tmp2[:, :], num_im[:, :], rim[:, :], ALU.mult)
        TT(q_re[:, :], tmp1[:, :], tmp2[:, :], ALU.add)
        TT(tmp1[:, :], num_im[:, :], rr[:, :], ALU.mult)
        TT(tmp2[:, :], num_re[:, :], rim[:, :], ALU.mult)
        TT(q_im[:, :], tmp1[:, :], tmp2[:, :], ALU.subtract)
        # Wc = (w_re + i*w_im) * q
        w_re = v3(wt[:, :], 2)[:, :, 0]
        w_im = v3(wt[:, :], 2)[:, :, 1]
        wc_re = t([P, G], "wc_re")
        wc_im = t([P, G], "wc_im")
        TT(tmp1[:, :], w_re, q_re[:, :], ALU.mult)
        TT(tmp2[:, :], w_im, q_im[:, :], ALU.mult)
        TT(wc_re[:, :], tmp1[:, :], tmp2[:, :], ALU.subtract)
        TT(tmp1[:, :], w_re, q_im[:, :], ALU.mult)
        TT(tmp2[:, :], w_im, q_re[:, :], ALU.mult)
        TT(wc_im[:, :], tmp1[:, :], tmp2[:, :], ALU.add)

        # ---- U = Wc * exp(a*B*j - pmax)  [P, G*J] ----
        EC = t([P, G * J], "EC")
        ES = t([P, G * J], "ES")
        TT(EC[:, :], Eu[:, :], Cu[:, :], ALU.mult)
        TT(ES[:, :], Eu[:, :], Su[:, :], ALU.mult)
        U_re = t([P, G * J], "U_re")
        U_im = t([P, G * J], "U_im")
        tmpu = t([P, G * J], "tmpu")
        TT(v3(U_re[:, :], J), v3(EC[:, :], J), bc(wc_re[:, :], J), ALU.mult)
        TT(v3(tmpu[:, :], J), v3(ES[:, :], J), bc(wc_im[:, :], J), ALU.mult)
        TT(U_re[:, :], U_re[:, :], tmpu[:, :], ALU.subtract)
        TT(v3(U_im[:, :], J), v3(ES[:, :], J), bc(wc_re[:, :], J), ALU.mult)
        TT(v3(tmpu[:, :], J), v3(EC[:, :], J), bc(wc_im[:, :], J), ALU.mult)
        TT(U_im[:, :], U_im[:, :], tmpu[:, :], ALU.add)

        # ---- V = exp(a*b)  [P, G*B]  (V_im negated) ----
        V_re = t([P, G * B], "V_re")
        V_im = t([P, G * B], "V_im")
        TT(V_re[:, :], Ev[:, :], Cv[:, :], ALU.mult)
        TT(V_im[:, :], Ev[:, :], Sv[:, :], ALU.mult)

        # ---- matmuls: K[h, j*B+b] = sum_n U*V, then store ----
        for g in range(G):
            ps = ppool.tile([P, B], f32, tag="ps", name=f"ps{g}")
            for h2 in (0, 1):
                rows = slice(64 * h2, 64 * (h2 + 1))
                nc.tensor.matmul(ctx, ps[rows, :],
                                 U_re[rows, g * J:(g + 1) * J],
                                 V_re[rows, g * B:(g + 1) * B],
                                 start=True, stop=False)
                nc.tensor.matmul(ctx, ps[rows, :],
                                 U_im[rows, g * J:(g + 1) * J],
                                 V_im[rows, g * B:(g + 1) * B],
                                 start=False, stop=True)
            ksb = kpool.tile([P, B], f32, tag="ksb", name=f"ksb{g}")
            nc.vector.tensor_copy(ksb[:, :], ps[:, :])
            nc.sync.dma_start(
                k_out[2 * g:2 * g + 2, :].rearrange("h (j b) -> (h j) b", b=B),
                ksb[:, :])

    nc.compile()
    return nc


def _get_compiled():
    if "nc" not in _COMPILED:
        _COMPILED["nc"] = _build()
    return _COMPILED["nc"]


def _shard_inputs(Lambda, log_dt, W):
    """Pure relayout of the inputs into the per-core partition layouts."""
    lam_b = np.concatenate([Lambda, Lambda], axis=0).astype(np.float32)  # [128,2]
    in_maps = []
    for core in range(NCORES):
        h0 = HLOC * core
        ld = log_dt[h0:h0 + HLOC].reshape(G, 2, 2)          # [g, h2, c]
        ldt_b = np.repeat(ld.transpose(1, 0, 2), 64, axis=0)  # [128, g, c]
        w = W[0, h0:h0 + HLOC].reshape(G, 2, N_STATE, 2)    # [g, h2, n, c]
        w_b = w.transpose(1, 2, 0, 3).reshape(P, 2 * G)     # [(h2 n), (g c)]
        in_maps.append({
            "lam_b": np.ascontiguousarray(lam_b),
            "ldt_b": np.ascontiguousarray(ldt_b.reshape(P, 2 * G)),
            "w_b": np.ascontiguousarray(w_b),
        })
    return in_maps


def kernel(**inputs) -> np.ndarray:
    from concourse.bass_utils import run_bass_kernel_spmd

    L = int(np.asarray(inputs["L"]))
    assert L == SEQ_LEN, f"kernel hardcodes L={SEQ_LEN}, got {L}"
    Lambda = np.ascontiguousarray(np.asarray(inputs["Lambda"], np.float32))
    log_dt = np.ascontiguousarray(np.asarray(inputs["log_dt"], np.float32))
    W = np.ascontiguousarray(np.asarray(inputs["W"], np.float32))
    assert Lambda.shape == (N_STATE, 2) and log_dt.shape == (H_DIM, 2)
    assert W.shape == (1, H_DIM, N_STATE, 2)

    nc = _get_compiled()
    in_maps = _shard_inputs(Lambda, log_dt, W)
    res = run_bass_kernel_spmd(nc, in_maps, list(range(NCORES)))
    K = np.concatenate([res.results[c]["k_out"] for c in range(NCORES)],
                       axis=0)[None, :, :]
    return np.ascontiguousarray(K.astype(np.float32))


# revision 8
# speedup vs baseline: 1.0984x; 1.0984x over previous
"""DSS ('softmax' variant, arxiv 2203.14343) kernel for Trainium2.

Computes K[c,h,l] = Re( sum_n Wc[c,h,n] * exp(dt_Lambda[h,n] * l - P_max[h,n]) )
with the structured-softmax normalization of Wc, for
C=1, H=128, N=64, L=8192, sharded over H across 8 NeuronCores.

Factorization, per core (16 h-channels -> 8 groups of 2h x 64n = 128
partitions):
  a[h,n]   = dt_re[h]*Lam_re[n] + i*dt_im[h]*Lam_im[n]      (dt_Lambda)
  l = j*B + b   (B=128, J=64)
  K[h,l]   = Re( sum_n (Wc * exp(a*B*j - pmax)) * exp(a*b) )
           = sum_n U_re*V_re - U_im*V_im        <- two PE matmul-accumulates
                                                   (contraction over n)
All imaginary phases are tracked in TURNS (x = theta/2pi) so that range
reduction is an exact magic-round + subtract, and sin/cos come from the HW
Sin2pi table (act set 22, exp_and_friends, which also holds Exp and Copy ->
exactly one activation-table load in the whole kernel). Sin2pi is not
exposed in bass: we emit Sin and patch the serialized BIR before the
neuron compile (kernel marks the instruction names).

Key exactness facts used:
  - rate pre-reduction is exact: sin2pi(r*b) for integer b is invariant
    under r -> r - round(r)
  - x*128, x*8192 are exact in f32 (power-of-two scales)
  - frac = x - round(x) is exact (Sterbenz); round via the 1.5*2^23 trick
"""
import numpy as np
from contextlib import ExitStack

H_DIM = 128
N_STATE = 64
SEQ_LEN = 8192
NCORES = 8
HLOC = H_DIM // NCORES      # 16 channels per core
G = HLOC // 2               # 8 groups of (2h x 64n) partitions
B = 128                     # intra-block length (matmul moving free dim)
J = SEQ_LEN // B            # 64 blocks (matmul stationary free dim / M)
P = 128
EPS = 1e-7
ACT_SET_EXP_AND_FRIENDS = 22

_COMPILED = {}


def _build(use_sin2pi=True):
    import concourse.bacc as bacc
    import concourse.tile as tile
    import concourse.mybir as mybir
    import orjson

    f32 = mybir.dt.float32
    AF = mybir.ActivationFunctionType
    ALU = mybir.AluOpType

    nc = bacc.Bacc("TRN2", target_bir_lowering=False, debug=False,
                   num_devices=NCORES)
    lam_in = nc.dram_tensor("lam_b", [P, 2], f32, kind="ExternalInput").ap()
    ldt_in = nc.dram_tensor("ldt_b", [P, 2 * G], f32, kind="ExternalInput").ap()
    w_in = nc.dram_tensor("w_b", [P, 2 * G], f32, kind="ExternalInput").ap()
    k_out = nc.dram_tensor("k_out", [HLOC, SEQ_LEN], f32,
                           kind="ExternalOutput").ap()

    M_MAGIC = float(np.float32(1.5 * 2 ** 23))
    INV2PI = float(np.float32(1.0 / (2 * np.pi)))
    sin_marks = []

    with tile.TileContext(nc) as tc, ExitStack() as ctx:
        pool = ctx.enter_context(tc.tile_pool(name="main", bufs=1))
        ppool = ctx.enter_context(tc.tile_pool(name="ps", bufs=G, space="PSUM"))
        kpool = ctx.enter_context(tc.tile_pool(name="ksb", bufs=G))

        def t(shape, tag):
            return pool.tile(shape, f32, tag=tag, name=tag)

        def v3(ap, inner):
            return ap.rearrange("p (g x) -> p g x", x=inner)

        def bc(ap, inner):
            return ap.to_broadcast([P, G, inner])

        def sin2pi(out, in_, **kw):
            """Emits Sin; BIR-patched to Sin2pi post-compile. For CoreSim
            the python-side instruction's scale is multiplied by 2*pi after
            the JSON freeze, matching sin2pi semantics in simulation."""
            bi = nc.scalar.activation(out, in_, AF.Sin, **kw)
            sin_marks.append(bi.ins)
            return bi

        TT = nc.vector.tensor_tensor
        TS = nc.vector.tensor_scalar
        STT = nc.vector.scalar_tensor_tensor
        GT = nc.gpsimd.tensor_tensor
        GS = nc.gpsimd.tensor_scalar

        # ---- loads ----
        lam = t([P, 2], "lam")
        ldt = t([P, 2 * G], "ldt")
        wt = t([P, 2 * G], "wt")
        nc.sync.dma_start(lam[:, :], lam_in)
        nc.sync.dma_start(ldt[:, :], ldt_in)
        nc.sync.dma_start(wt[:, :], w_in)
        lam_re, lam_im = lam[:, 0:1], lam[:, 1:2]

        # ---- iotas / consts ----
        ij1 = t([P, J], "ij1")      # 0..63, for U exps (per-group ACT scale)
        nc.gpsimd.iota(ij1[:, :], pattern=[[1, J]], base=0,
                       channel_multiplier=0,
                       allow_small_or_imprecise_dtypes=True)
        ib1 = t([P, B], "ib1")      # 0..127, for V exps
        nc.gpsimd.iota(ib1[:, :], pattern=[[1, B]], base=0,
                       channel_multiplier=0,
                       allow_small_or_imprecise_dtypes=True)
        iota_j = t([P, G * J], "iota_j")   # g-blocked for phase tensors
        nc.gpsimd.iota(iota_j[:, :], pattern=[[0, G], [1, J]], base=0,
                       channel_multiplier=0,
                       allow_small_or_imprecise_dtypes=True)
        iota_b = t([P, G * B], "iota_b")
        nc.gpsimd.iota(iota_b[:, :], pattern=[[0, G], [1, B]], base=0,
                       channel_multiplier=0,
                       allow_small_or_imprecise_dtypes=True)
        quarter = t([P, 1], "quarter")
        nc.gpsimd.memset(quarter[:, :], 0.25)
        qcol = quarter[:, 0:1]

        # ---- ACT: dt = exp(log_dt) ----
        dt = t([P, 2 * G], "dt")
        nc.scalar.activation(dt[:, :], ldt[:, :], AF.Exp)
        dt_re = v3(dt[:, :], 2)[:, :, 0]
        dt_im = v3(dt[:, :], 2)[:, :, 1]

        # ---- per-(h,n) scalars, [P, G] (small ops -> gpsimd where easy) ----
        lam_im_t = t([P, 1], "lam_im_t")   # Lam_im / 2pi
        GS(lam_im_t[:, :], lam_im, INV2PI, None, ALU.mult)
        a_re = t([P, G], "a_re")
        a_t = t([P, G], "a_t")             # dt_Lambda imag, in turns
        GS(a_re[:, :], dt_re, lam_re, None, ALU.mult)
        GS(a_t[:, :], dt_im, lam_im_t[:, 0:1], None, ALU.mult)
        gt0 = t([P, 1], "gt0")
        GS(gt0[:, :], lam_re, 0.0, None, ALU.is_gt)
        sgn = t([P, 1], "sgn")
        GS(sgn[:, :], gt0[:, :], -2.0, 1.0, ALU.mult, ALU.add)
        dn_re = t([P, G], "dn_re")
        dn_t = t([P, G], "dn_t")
        GS(dn_re[:, :], a_re[:, :], sgn[:, 0:1], None, ALU.mult)
        GS(dn_t[:, :], a_t[:, :], sgn[:, 0:1], None, ALU.mult)
        dnL_t = t([P, G], "dnL_t")         # exact: power-of-two scale
        GS(dnL_t[:, :], dn_t[:, :], float(SEQ_LEN), None, ALU.mult)
        pm_re = t([P, G], "pm_re")
        pm_t = t([P, G], "pm_t")
        GS(pm_re[:, :], a_re[:, :], gt0[:, 0:1], float(SEQ_LEN - 1),
           ALU.mult, ALU.mult)
        GS(pm_t[:, :], a_t[:, :], gt0[:, 0:1], float(SEQ_LEN - 1),
           ALU.mult, ALU.mult)
        npm_re = t([P, G], "npm_re")
        GS(npm_re[:, :], pm_re[:, :], -1.0, None, ALU.mult)
        a_reB = t([P, G], "a_reB")
        GS(a_reB[:, :], a_re[:, :], float(B), None, ALU.mult)

        def frac_small(x, name, shift=None):
            """x - round(x [+ shift]) on gpsimd, [P, G] tiles."""
            k = t([P, G], name + "_k")
            if shift is None:
                GS(k[:, :], x, M_MAGIC, M_MAGIC, ALU.add, ALU.subtract)
                r = t([P, G], name + "_r")
                GT(r[:, :], x, k[:, :], ALU.subtract)
            else:
                xs = t([P, G], name + "_xs")
                GS(xs[:, :], x, shift, None, ALU.add)
                GS(k[:, :], xs[:, :], M_MAGIC, M_MAGIC, ALU.add, ALU.subtract)
                r = t([P, G], name + "_r")
                GT(r[:, :], xs[:, :], k[:, :], ALU.subtract)
            return r

        # pre-reduced rates (exact for integer multipliers)
        ar_t = frac_small(a_t[:, :], "ar")           # |.| <= 0.5
        aB_t = t([P, G], "aB_t")
        GS(aB_t[:, :], a_t[:, :], float(B), None, ALU.mult)   # exact
        arB_t = frac_small(aB_t[:, :], "arB")
        # sin/cos args for the normalization scalars
        fs_d = frac_small(dn_t[:, :], "fsd")
        fc_d = frac_small(dn_t[:, :], "fcd", shift=0.25)
        fs_dL = frac_small(dnL_t[:, :], "fsdL")
        fc_dL = frac_small(dnL_t[:, :], "fcdL", shift=0.25)

        # ---- phase tensors in turns (DVE, broadcast APs) ----
        thu_t = t([P, G * J], "thu_t")
        TT(v3(thu_t[:, :], J), v3(iota_j[:, :], J), bc(arB_t[:, :], J),
           ALU.mult)
        TT(v3(thu_t[:, :], J), v3(thu_t[:, :], J), bc(pm_t[:, :], J),
           ALU.subtract)
        thv_t = t([P, G * B], "thv_t")
        TT(v3(thv_t[:, :], B), v3(iota_b[:, :], B), bc(ar_t[:, :], B),
           ALU.mult)

        # frac-reduce the phase tensors (sin arg; cos via +0.25 frac)
        def frac_big(x, w, name, shift=None):
            src = x
            if shift is not None:
                xs = t([P, w], name + "_xs")
                TS(xs[:, :], x, shift, None, ALU.add)
                src = xs[:, :]
            k = t([P, w], name + "_k")
            TS(k[:, :], src, M_MAGIC, M_MAGIC, ALU.add, ALU.subtract)
            r = t([P, w], name + "_r")
            TT(r[:, :], src, k[:, :], ALU.subtract)
            return r

        fs_u = frac_big(thu_t[:, :], G * J, "fsu")
        fc_u = frac_big(thu_t[:, :], G * J, "fcu", shift=0.25)
        fs_v = frac_big(thv_t[:, :], G * B, "fsv")
        fc_v = frac_big(thv_t[:, :], G * B, "fcv", shift=0.25)

        # ---- ACT batch: exps (per-group scale/bias trick) + sin2pi ----
        e1 = t([P, G], "e1")
        eL = t([P, G], "eL")
        nc.scalar.activation(e1[:, :], dn_re[:, :], AF.Exp)
        nc.scalar.activation(eL[:, :], dn_re[:, :], AF.Exp,
                             scale=float(SEQ_LEN))
        Eu = t([P, G * J], "Eu")
        Ev = t([P, G * B], "Ev")
        for g in range(G):
            nc.scalar.activation(Eu[:, g * J:(g + 1) * J], ij1[:, :], AF.Exp,
                                 scale=a_reB[:, g:g + 1],
                                 bias=npm_re[:, g:g + 1])
            nc.scalar.activation(Ev[:, g * B:(g + 1) * B], ib1[:, :], AF.Exp,
                                 scale=a_re[:, g:g + 1])

        c1 = t([P, G], "c1")
        s1 = t([P, G], "s1")
        cL = t([P, G], "cL")
        sL = t([P, G], "sL")
        sin2pi(c1[:, :], fc_d[:, :])
        sin2pi(s1[:, :], fs_d[:, :])
        sin2pi(cL[:, :], fc_dL[:, :])
        sin2pi(sL[:, :], fs_dL[:, :])
        Cu = t([P, G * J], "Cu")
        Su = t([P, G * J], "Su")
        sin2pi(Cu[:, :], fc_u[:, :])
        sin2pi(Su[:, :], fs_u[:, :])
        Cv = t([P, G * B], "Cv")
        Sv = t([P, G * B], "Sv")   # sin2pi(-x) = -sin2pi(x): matmul minus
        sin2pi(Cv[:, :], fc_v[:, :])
        sin2pi(Sv[:, :], fs_v[:, :], scale=-1.0)

        # ---- normalization coefficients Wc (all [P, G]) ----
        num_re = t([P, G], "num_re")
        num_im = t([P, G], "num_im")
        TT(num_re[:, :], e1[:, :], c1[:, :], ALU.mult)
        TS(num_re[:, :], num_re[:, :], -1.0, None, ALU.add)
        TT(num_im[:, :], e1[:, :], s1[:, :], ALU.mult)
        den_re = t([P, G], "den_re")
        den_im = t([P, G], "den_im")
        TT(den_re[:, :], eL[:, :], cL[:, :], ALU.mult)
        TS(den_re[:, :], den_re[:, :], -1.0, None, ALU.add)
        TT(den_im[:, :], eL[:, :], sL[:, :], ALU.mult)
        neg_lam_im = t([P, 1], "neg_lam_im")
        TS(neg_lam_im[:, :], lam_im, -1.0, None, ALU.mult)
        x_re = t([P, G], "x_re")
        x_im = t([P, G], "x_im")
        tmp1 = t([P, G], "tmp1")
        tmp2 = t([P, G], "tmp2")
        TS(tmp1[:, :], den_re[:, :], lam_re, None, ALU.mult)
        STT(x_re[:, :], den_im[:, :], neg_lam_im[:, 0:1], tmp1[:, :],
            ALU.mult, ALU.add)
        TS(tmp1[:, :], den_im[:, :], lam_re, None, ALU.mult)
        STT(x_im[:, :], den_re[:, :], lam_im, tmp1[:, :], ALU.mult, ALU.add)
        d = t([P, G], "d")
        TT(d[:, :], x_re[:, :], x_re[:, :], ALU.mult)
        TT(tmp1[:, :], x_im[:, :], x_im[:, :], ALU.mult)
        TT(d[:, :], d[:, :], tmp1[:, :], ALU.add)
        TS(d[:, :], d[:, :], float(EPS), None, ALU.add)
        inv = t([P, G], "inv")
        nc.vector.reciprocal(inv[:, :], d[:, :])
        rr = t([P, G], "rr")
        rim = t([P, G], "rim")  # rim = x_im*inv = -recip_im
        TT(rr[:, :], x_re[:, :], inv[:, :], ALU.mult)
        TT(rim[:, :], x_im[:, :], inv[:, :], ALU.mult)
        q_re = t([P, G], "q_re")
        q_im = t([P, G], "q_im")
        TT(tmp1[:, :], num_re[:, :], rr[:, :], ALU.mult)
        TT(tmp2[:, :], num_im[:, :], rim[:, :], ALU.mult)
        TT(q_re[:, :], tmp1[:, :], tmp2[:, :], ALU.add)
        TT(tmp1[:, :], num_im[:, :], rr[:, :], ALU.mult)
        TT(tmp2[:, :], num_re[:, :], rim[:, :], ALU.mult)
        TT(q_im[:, :], tmp1[:, :], tmp2[:, :], ALU.subtract)
        w_re = v3(wt[:, :], 2)[:, :, 0]
        w_im = v3(wt[:, :], 2)[:, :, 1]
        wc_re = t([P, G], "wc_re")
        wc_im = t([P, G], "wc_im")
        TT(tmp1[:, :], w_re, q_re[:, :], ALU.mult)
        TT(tmp2[:, :], w_im, q_im[:, :], ALU.mult)
        TT(wc_re[:, :], tmp1[:, :], tmp2[:, :], ALU.subtract)
        TT(tmp1[:, :], w_re, q_im[:, :], ALU.mult)
        TT(tmp2[:, :], w_im, q_re[:, :], ALU.mult)
        TT(wc_im[:, :], tmp1[:, :], tmp2[:, :], ALU.add)

        # ---- U = Wc * exp(a*B*j - pmax)  [P, G*J] ----
        EC = t([P, G * J], "EC")
        ES = t([P, G * J], "ES")
        TT(EC[:, :], Eu[:, :], Cu[:, :], ALU.mult)
        TT(ES[:, :], Eu[:, :], Su[:, :], ALU.mult)
        U_re = t([P, G * J], "U_re")
        U_im = t([P, G * J], "U_im")
        tmpu = t([P, G * J], "tmpu")
        TT(v3(U_re[:, :], J), v3(EC[:, :], J), bc(wc_re[:, :], J), ALU.mult)
        TT(v3(tmpu[:, :], J), v3(ES[:, :], J), bc(wc_im[:, :], J), ALU.mult)
        TT(U_re[:, :], U_re[:, :], tmpu[:, :], ALU.subtract)
        TT(v3(U_im[:, :], J), v3(ES[:, :], J), bc(wc_re[:, :], J), ALU.mult)
        TT(v3(tmpu[:, :], J), v3(EC[:, :], J), bc(wc_im[:, :], J), ALU.mult)
        TT(U_im[:, :], U_im[:, :], tmpu[:, :], ALU.add)

        # ---- V = exp(a*b)  [P, G*B]  (V_im negated) ----
        V_re = t([P, G * B], "V_re")
        V_im = t([P, G * B], "V_im")
        TT(V_re[:, :], Ev[:, :], Cv[:, :], ALU.mult)
        TT(V_im[:, :], Ev[:, :], Sv[:, :], ALU.mult)

        # ---- matmuls: K[h, j*B+b] = sum_n U*V, copy out via ACT ----
        for g in range(G):
            ps = ppool.tile([P, B], f32, tag="ps", name=f"ps{g}")
            for h2 in (0, 1):
                rows = slice(64 * h2, 64 * (h2 + 1))
                nc.tensor.matmul(ps[rows, :],
                                 U_re[rows, g * J:(g + 1) * J],
                                 V_re[rows, g * B:(g + 1) * B],
                                 start=True, stop=False)
                nc.tensor.matmul(ps[rows, :],
                                 U_im[rows, g * J:(g + 1) * J],
                                 V_im[rows, g * B:(g + 1) * B],
                                 start=False, stop=True)
            ksb = kpool.tile([P, B], f32, tag="ksb", name=f"ksb{g}")
            nc.scalar.copy(ksb[:, :], ps[:, :])
            nc.sync.dma_start(
                k_out[2 * g:2 * g + 2, :].rearrange("h (j b) -> (h j) b", b=B),
                ksb[:, :])

    nc.compile()

    if use_sin2pi:
        j = orjson.loads(nc.to_json_bytes())
        marks = {i.name for i in sin_marks}
        n_func = n_load = 0
        for f in j["functions"]:
            for blk in f["blocks"]:
                for ins in blk["instructions"]:
                    if ins.get("func") and ins.get("name") in marks:
                        ins["func"] = "Sin2pi"
                        n_func += 1
                    if ins.get("opcode") == "LoadActFuncSet":
                        ins["act_func_set_id"] = ACT_SET_EXP_AND_FRIENDS
                        n_load += 1
        assert n_func == len(sin_marks), (n_func, len(sin_marks))
        patched = orjson.dumps(j)
        nc.to_json_bytes = lambda: patched
        # CoreSim compatibility: fold the 2*pi into the python-side scale so
        # simulating the (unpatched-semantics) Sin matches HW Sin2pi.
        two_pi = float(np.float32(2 * np.pi))
        for mi in sin_marks:
            old = mi.ins[2]
            assert isinstance(old, mybir.ImmediateValue), old
            mi.ins[2] = mybir.ImmediateValue(dtype=mybir.dt.float32,
                                             value=old.value * two_pi)
    return nc


def _get_compiled():
    if "nc" not in _COMPILED:
        _COMPILED["nc"] = _build()
    return _COMPILED["nc"]


def _shard_inputs(Lambda, log_dt, W):
    """Pure relayout of the inputs into the per-core partition layouts."""
    lam_b = np.concatenate([Lambda, Lambda], axis=0).astype(np.float32)  # [128,2]
    in_maps = []
    for core in range(NCORES):
        h0 = HLOC * core
        ld = log_dt[h0:h0 + HLOC].reshape(G, 2, 2)          # [g, h2, c]
        ldt_b = np.repeat(ld.transpose(1, 0, 2), 64, axis=0)  # [128, g, c]
        w = W[0, h0:h0 + HLOC].reshape(G, 2, N_STATE, 2)    # [g, h2, n, c]
        w_b = w.transpose(1, 2, 0, 3).reshape(P, 2 * G)     # [(h2 n), (g c)]
        in_maps.append({
            "lam_b": np.ascontiguousarray(lam_b),
            "ldt_b": np.ascontiguousarray(ldt_b.reshape(P, 2 * G)),
            "w_b": np.ascontiguousarray(w_b),
        })
    return in_maps


def kernel(**inputs) -> np.ndarray:
    from concourse.bass_utils import run_bass_kernel_spmd

    L = int(np.asarray(inputs["L"]))
    assert L == SEQ_LEN, f"kernel hardcodes L={SEQ_LEN}, got {L}"
    Lambda = np.ascontiguousarray(np.asarray(inputs["Lambda"], np.float32))
    log_dt = np.ascontiguousarray(np.asarray(inputs["log_dt"], np.float32))
    W = np.ascontiguousarray(np.asarray(inputs["W"], np.float32))
    assert Lambda.shape == (N_STATE, 2) and log_dt.shape == (H_DIM, 2)
    assert W.shape == (1, H_DIM, N_STATE, 2)

    nc = _get_compiled()
    in_maps = _shard_inputs(Lambda, log_dt, W)
    res = run_bass_kernel_spmd(nc, in_maps, list(range(NCORES)))
    K = np.concatenate([res.results[c]["k_out"] for c in range(NCORES)],
                       axis=0)[None, :, :]
    return np.ascontiguousarray(K.astype(np.float32))
